# revision 68
# baseline (speedup 1.0000x reference)
"""Trainium2 Bass kernel for nn_Attention_54245436948569.

Full multi-head attention (qkv proj + interleaved RoPE + softmax attention +
out proj) for B=2, N=2048, D=1024, H=16, DH=64, sharded over 8 NeuronCores as
(batch x head-group): core c handles batch c//4 and heads [4*(c%4), 4*(c%4)+4).

Per-core kernel computes a row-parallel partial of the out-projection
([2048, 1024] fp32); the host sums the 4 partials per batch and adds b_out.

The scalar engine's exp stream (128 x ~1.33us ACTIVATE over [128,1024] psum
tiles) is the ~171us floor; the PE's matmul stream (~185us of issue time) is
just above it.  Key scheduling decisions (vs the naive per-ktile loop):

- The emission order software-pipelines the per-ktile scores -> exp -> AV
  chain: sc(kt+1) is emitted BEFORE av(kt-2) and the extras (lag-2 AV), and
  the next block's sc(0) before av(15)/norm, so the PE's in-order stream
  never makes the scalar engine wait on an un-issued scores matmul, and
  DMA-gated extras cannot starve the exp stream.
- Softmax denominators ride the AV matmul as a leading ones-column (psum
  partition 0); normalization reciprocal+broadcast runs on the idle gpsimd
  engine (partition_broadcast reads the TILE's partition 0 on HW -- its slice
  base is ignored), with the multiply deferred into the next block so the
  DVE's strict FIFO never waits on the gpsimd latency.  The last block uses
  a K=1 PE-matmul broadcast instead (lower latency on the tail).
- Each dma_start costs ~0.6-1us of issue time on its engine and a queue
  sustains only ~100GB/s, so inputs are few large transfers spread over the
  three DMA queues (scalar/sync/gpsimd) with per-queue FIFO keeping each
  queue's first-exp-critical share (wqk / xT chunk 0 / rope tables) ahead of
  its wave-2 share.  Output and stkn DMAs split across sync+gpsimd.
- Warmup matmuls bracket the first projection so the HAM clock gate stays
  open (cold PE runs at 1.2GHz) through the DMA-paced gaps.
- qkv/v/out-proj work is spread as per-ktile "extras" with hand-placed
  deadlines; block 0 is PE-oversubscribed (it must produce all 16 V tiles),
  which costs ~10us of scalar idle that cannot move elsewhere (psum-bank
  aliasing pins each block's AV accumulators to a 1-block lifetime).
"""

import numpy as np
import ml_dtypes

B, N, D = 2, 2048, 1024
H, DH = 16, 64
THETA = 10000.0

BF = ml_dtypes.bfloat16

_CACHE = {}


def _build():
    from contextlib import ExitStack
    import concourse.mybir as mybir
    import concourse.tile as tile
    from concourse import bacc
    from concourse.compiler_utils import get_compiler_flags, set_compiler_flags

    set_compiler_flags([f.replace("--enable-ldw-opt=false", "--enable-ldw-opt=true")
                        for f in get_compiler_flags()])
    # NOTE: bass_utils hardcodes --enable-ldw-opt=false in its walrus argv
    # (the flag replacement above never reaches that path).  Forcing it true
    # there was tried: walrus codegen rejects this kernel's LDWEIGHTS forms
    # (visitInstLdweights error), so it must stay off.

    FP32 = mybir.dt.float32
    F32R = mybir.dt.float32r
    BF16 = mybir.dt.bfloat16
    AF = mybir.ActivationFunctionType
    MUL = mybir.AluOpType.mult
    ADD = mybir.AluOpType.add

    nc = bacc.Bacc(None, target_bir_lowering=False)

    NT = N // 512            # 4 token 512-blocks
    KT_D = D // 128          # 8 contraction tiles for qkv
    KT_N = N // 128          # 16 k-token tiles for attention
    SCALE = 1.0 / float(np.sqrt(DH))

    with tile.TileContext(nc) as tc:
        with tc.tile_pool(name="dram", bufs=1, space="DRAM") as dram:
            xT_d = dram.tile([NT, 128, KT_D, 512], BF16, kind="ExternalInput", name="xT", uniquify=False)
            wqk_d = dram.tile([128, KT_D, 512], BF16, kind="ExternalInput", name="wqk", uniquify=False)
            wv_d = dram.tile([128, KT_D, 256], BF16, kind="ExternalInput", name="wv", uniquify=False)
            wo_d = dram.tile([128, 2, 1024], BF16, kind="ExternalInput", name="wo", uniquify=False)
            cs_d = dram.tile([64, 2, N], BF16, kind="ExternalInput", name="cs", uniquify=False)
            out_d = dram.tile([KT_N, 128, D], BF16, kind="ExternalOutput", name="out", uniquify=False)

            ctx = ExitStack()
            const = ctx.enter_context(tc.tile_pool(name="const", bufs=1))
            ropep = ctx.enter_context(tc.tile_pool(name="ropep", bufs=4))
            attnp = ctx.enter_context(tc.tile_pool(name="attnp", bufs=24))
            stkp = ctx.enter_context(tc.tile_pool(name="stkp", bufs=6))
            normp = ctx.enter_context(tc.tile_pool(name="normp", bufs=3))
            outp = ctx.enter_context(tc.tile_pool(name="outp", bufs=3))
            # PSUM budget (8 banks): misc 2x1, scores 2x2, av 2x1
            ps_misc = ctx.enter_context(tc.tile_pool(name="ps_misc", bufs=2, space="PSUM"))
            ps_sc = ctx.enter_context(tc.tile_pool(name="ps_sc", bufs=2, space="PSUM"))
            ps_av = ctx.enter_context(tc.tile_pool(name="ps_av", bufs=2, space="PSUM"))

            # ---- persistent SBUF tensors ----
            wqk = const.tile([128, KT_D, 512], BF16)
            wv = const.tile([128, KT_D, 256], BF16)
            wo = const.tile([128, 2, 1024], BF16)
            cs = const.tile([128, 2, N], BF16)
            cos2 = cs[:, 0, :]
            sin2n = cs[:, 1, :]
            q2n = [[const.tile([128, 512], BF16, name=f"q2_{p}_{nt}") for nt in range(NT)] for p in range(2)]
            k2n = [[const.tile([128, 512], BF16, name=f"k2_{p}_{nt}") for nt in range(NT)] for p in range(2)]
            # col 0 = ones (softmax denominator rides the AV matmul on psum
            # partition 0, where gpsimd partition_broadcast can read it)
            v_t = [const.tile([128, 4, 65], BF16, name=f"v_{tt}") for tt in range(KT_N)]
            ones1f = const.tile([128, 72], FP32)

            xT_nt = [const.tile([128, KT_D, 512], BF16, name=f"xT_{nt}") for nt in range(NT)]

            # ---- wave-1 DMAs: critical path for the first exp.  Few, large
            # transfers: each dma_start costs ~0.6-1us of engine issue time ----
            with nc.named_scope("load"):
                # wave-1: first-exp-critical transfers (wqk, xT0, rope tables)
                # in halves so the first projection matmuls can start before
                # the whole transfer lands
                nc.scalar.dma_start(out=wqk[:, 0:4, :], in_=wqk_d[:, 0:4, :])
                nc.scalar.dma_start(out=wqk[:, 4:8, :], in_=wqk_d[:, 4:8, :])
                nc.sync.dma_start(out=xT_nt[0][:, 0:4, :], in_=xT_d[0, :, 0:4, :])
                nc.sync.dma_start(out=xT_nt[0][:, 4:8, :], in_=xT_d[0, :, 4:8, :])
                # rope tables: rows repeat with period 64 -> land them twice
                nc.gpsimd.dma_start(out=cs[0:64], in_=cs_d[:])
                nc.gpsimd.dma_start(out=cs[64:128], in_=cs_d[:])

                # preload the exp table set while DMAs are in flight
                warm8 = const.tile([128, 8], FP32)
                nc.vector.memset(warm8[:], 1.0)
                warmup = const.tile([128, 8], FP32)
                nc.scalar.activation(warmup[:], warm8[:], AF.Exp, scale=0.125)
                nc.vector.memset(ones1f[:], 1.0)
                for tt in range(KT_N):
                    nc.vector.memset(v_t[tt][:, :, 0:1], 1.0)
                # warm the PE (HAM clock gate) during the DMA-wait window so
                # the first projection matmuls run at 2.4GHz
                wsrc = const.tile([128, 512], BF16)
                nc.vector.memset(wsrc[:], 0.5)
                pw = ps_misc.tile([128, 512], FP32, tag="misc", name="pw")
                for r in range(10):
                    nc.tensor.matmul(pw[:], wsrc[:, 0:128], wsrc[:],
                                     start=(r == 0), stop=(r == 9))
                # wave-2 DMAs balanced across the sync/gpsimd queues; per-queue
                # FIFO keeps them behind that queue's wave-1 share.  xT1 (the
                # first extras deadline, block-0 kt1) is split across queues.
                # NOTHING else goes on the scalar queue: a dma_start can block
                # its issuing engine on ring credits, and the scalar engine's
                # FIFO feeds the exp stream.
                nc.gpsimd.dma_start(out=wv[:], in_=wv_d[:])
                nc.gpsimd.dma_start(out=xT_nt[1][:, 0:4, :], in_=xT_d[1, :, 0:4, :])
                nc.sync.dma_start(out=xT_nt[1][:, 4:8, :], in_=xT_d[1, :, 4:8, :])
                nc.sync.dma_start(out=xT_nt[2][:], in_=xT_d[2])
                nc.gpsimd.dma_start(out=xT_nt[3][:], in_=xT_d[3])
                nc.gpsimd.dma_start(out=wo[:], in_=wo_d[:])



            pair_mask = []
            for i in range(16):
                pair_mask += [2 * i + 1, 2 * i]

            # ---- building blocks ----
            def qk_rope(m, nt, pqk, evict_engine):
                dest = (q2n if m < 2 else k2n)[m % 2][nt]
                ts = slice(nt * 512, (nt + 1) * 512)
                qraw = ropep.tile([128, 512], BF16, name="qraw")
                if evict_engine == "scalar":
                    nc.scalar.activation(qraw[:], pqk[:], AF.Copy)
                else:
                    nc.vector.tensor_copy(qraw[:], pqk[:])
                qcos = ropep.tile([128, 512], BF16, name="qcos")
                qsw = ropep.tile([128, 512], BF16, name="qsw")
                tmp = ropep.tile([128, 512], BF16, name="tmp")
                nc.vector.tensor_tensor(out=qcos[:], in0=qraw[:], in1=cos2[:, ts], op=MUL)
                nc.vector.stream_shuffle(qsw[:], qraw[:], pair_mask)
                nc.vector.tensor_tensor(out=tmp[:], in0=qsw[:], in1=sin2n[:, ts], op=MUL)
                nc.vector.tensor_tensor(out=dest[:], in0=qcos[:], in1=tmp[:], op=ADD)

            def qk_proj_pieces(m, nt, evict_engine):
                """4 closures, each emitting 2 of the 8 qkv matmuls (the last
                also emits the RoPE chain)."""
                state = {}

                def piece(i):
                    def run():
                        if i == 0:
                            state["pqk"] = ps_misc.tile([128, 512], FP32, tag="misc", name="pqk")
                        for kt in range(2 * i, 2 * i + 2):
                            nc.tensor.matmul(
                                state["pqk"][:],
                                wqk[:, kt, m * 128:(m + 1) * 128],
                                xT_nt[nt][:, kt, :],
                                start=(kt == 0), stop=(kt == KT_D - 1),
                            )
                        if i == 3:
                            qk_rope(m, nt, state["pqk"], evict_engine)
                    return run
                return [piece(i) for i in range(4)]

            def v_proj(tt):
                pv = ps_misc.tile([128, 512], FP32, tag="misc", name="pv")
                for kt in range(KT_D):
                    nc.tensor.matmul(
                        pv[:, 0:256],
                        xT_nt[tt // 4][:, kt, (tt % 4) * 128:(tt % 4 + 1) * 128],
                        wv[:, kt, :],
                        start=(kt == 0), stop=(kt == KT_D - 1),
                    )
                nc.vector.tensor_copy(v_t[tt][:, :, 1:65], pv[:, 0:256].rearrange("p (h d) -> p h d", d=64))

            def make_block(p, qb):
                st = {"attnT": {}}

                def sc_exp(kt):
                    with nc.named_scope(f"scores_p{p}_qb{qb}"):
                        pg = ps_sc.tile([128, 2, 512], FP32, tag="pg", name="pg")
                        attnT = attnp.tile([128, 2, 512], BF16, tag="attnT", name="attnT")
                        st["attnT"][kt] = attnT
                        knt, ko = kt // 4, (kt % 4) * 128
                        nc.tensor.matmul(
                            pg[:, 0, :], k2n[p][knt][0:64, ko:ko + 128], q2n[p][qb][0:64, :],
                            start=True, stop=True, tile_position=(0, 0),
                        )
                        nc.tensor.matmul(
                            pg[:, 1, :], k2n[p][knt][64:128, ko:ko + 128], q2n[p][qb][64:128, :],
                            start=True, stop=True, tile_position=(64, 0),
                        )
                        nc.scalar.activation(attnT[:], pg[:], AF.Exp, scale=SCALE)

                def av(kt):
                    with nc.named_scope(f"scores_p{p}_qb{qb}"):
                        if kt == 0:
                            st["pav_a"] = ps_av.tile([128, 512], FP32, tag="pav", name="pav_a")
                            st["pav_b"] = ps_av.tile([128, 512], FP32, tag="pav", name="pav_b")
                        attnT = st["attnT"].pop(kt)
                        nc.tensor.matmul(
                            st["pav_a"][0:65, :], v_t[kt][:, 2 * p, :], attnT[:, 0, :],
                            start=(kt == 0), stop=(kt == KT_N - 1),
                        )
                        nc.tensor.matmul(
                            st["pav_b"][0:65, :], v_t[kt][:, 2 * p + 1, :], attnT[:, 1, :],
                            start=(kt == 0), stop=(kt == KT_N - 1),
                        )

                def norm_front(last=False):
                    pav_a, pav_b = st["pav_a"], st["pav_b"]
                    with nc.named_scope(f"norm_p{p}_qb{qb}"):
                        st["ua"] = stkp.tile([128, 512], BF16, name="ua", bufs=2)
                        st["ub"] = stkp.tile([128, 512], BF16, name="ub", bufs=2)
                        sums = normp.tile([128, 1024], F32R if last else FP32, name="sums")
                        st["recipa"] = normp.tile([128, 512], FP32, name="recipa", bufs=2)
                        st["recipc"] = normp.tile([128, 512], FP32, name="recipc", bufs=2)
                        # evict the AV accumulators first so the next block's
                        # AV (aliasing these psum banks) is released quickly.
                        # On the tail the scalar engine is idle: split the
                        # copies across scalar and vector to halve the latency.
                        nc.vector.tensor_copy(sums[0:1, 0:512], pav_a[0:1, :])
                        nc.vector.tensor_copy(sums[0:1, 512:1024], pav_b[0:1, :])
                        if last:
                            nc.scalar.activation(st["ua"][0:65, :], pav_a[0:65, :], AF.Copy)
                            nc.scalar.activation(st["ub"][0:65, :], pav_b[0:65, :], AF.Copy)
                        else:
                            nc.vector.tensor_copy(st["ua"][0:65, :], pav_a[0:65, :])
                            nc.vector.tensor_copy(st["ub"][0:65, :], pav_b[0:65, :])
                        if last:
                            # latency-critical tail: broadcast the denominators
                            # across partitions with K=1 PE matmuls (the PE and
                            # psum are idle here, and this avoids the gpsimd
                            # launch+drain latency)
                            pbc = ps_misc.tile([128, 512], FP32, tag="misc", name="pbc")
                            nc.tensor.matmul(
                                pbc[0:65, :], ones1f[0:1, 0:65].bitcast(F32R),
                                sums[0:1, 0:512],
                                start=True, stop=True, tile_position=(0, 0),
                            )
                            pbc2 = ps_misc.tile([128, 512], FP32, tag="misc", name="pbc2")
                            nc.tensor.matmul(
                                pbc2[0:65, :], ones1f[0:1, 0:65].bitcast(F32R),
                                sums[0:1, 512:1024],
                                start=True, stop=True, tile_position=(0, 0),
                            )
                            nc.vector.reciprocal_approx_fast(out=st["recipa"][0:65, :], in_=pbc[0:65, :])
                            nc.vector.reciprocal_approx_fast(out=st["recipc"][0:65, :], in_=pbc2[0:65, :])
                        else:
                            # off the critical path: reciprocal on the 1-row
                            # sums, broadcast on the (otherwise idle) gpsimd
                            rrow = normp.tile([128, 1024], FP32, name="rrow")
                            nc.vector.reciprocal_approx_fast(out=rrow[0:1, :], in_=sums[0:1, :])
                            nc.gpsimd.partition_broadcast(st["recipa"][0:65, :], rrow[0:1, 0:512])
                            nc.gpsimd.partition_broadcast(st["recipc"][0:65, :], rrow[0:1, 512:1024])

                def norm_fin():
                    # emitted a few ktiles later so the DVE (strict FIFO) never
                    # actually waits on the gpsimd broadcast latency
                    with nc.named_scope(f"norm_p{p}_qb{qb}"):
                        stkn = stkp.tile([128, 512], BF16, name="stkn")
                        tmpn = stkp.tile([128, 512], BF16, name="tmpn", bufs=2)
                        nc.vector.tensor_tensor(out=tmpn[0:65, :], in0=st["ua"][0:65, :], in1=st["recipa"][0:65, :], op=MUL)
                        nc.sync.dma_start(out=stkn[0:64, :], in_=tmpn[1:65, :])
                        tmpm = stkp.tile([128, 512], BF16, name="tmpm", bufs=2)
                        nc.vector.tensor_tensor(out=tmpm[0:65, :], in0=st["ub"][0:65, :], in1=st["recipc"][0:65, :], op=MUL)
                        nc.gpsimd.dma_start(out=stkn[64:128, :], in_=tmpm[1:65, :])
                    return stkn

                return sc_exp, av, norm_front, norm_fin

            oproj_states = {}

            def oproj_piece(qb, qs, dt, act_evict=False, alt_pool=False):
                with nc.named_scope(f"oproj_qb{qb}"):
                    state = oproj_states[qb, qs]
                    if dt == 0:
                        state["ostg"] = outp.tile([128, 1024], BF16, name="ostg")
                    # on the tail the scores pool is idle: alternate psum pools
                    # for a 4-deep out-projection pipeline
                    pool = ps_sc if (alt_pool and (qs + dt) % 2) else ps_misc
                    tag = "pg" if (alt_pool and (qs + dt) % 2) else "misc"
                    po = pool.tile([128, 512], FP32, tag=tag, name="po")
                    stacked = state["stacked"]
                    ostg = state["ostg"]
                    for p in range(2):
                        nc.tensor.matmul(
                            po[:],
                            stacked[p][:, qs * 128:(qs + 1) * 128],
                            wo[:, p, dt * 512:(dt + 1) * 512],
                            start=(p == 0), stop=(p == 1),
                        )
                    if act_evict:
                        nc.scalar.activation(ostg[:, dt * 512:(dt + 1) * 512], po[:], AF.Copy)
                    else:
                        nc.vector.tensor_copy(ostg[:, dt * 512:(dt + 1) * 512], po[:])
                    dq = nc.sync if (qs + dt) % 2 == 0 else nc.gpsimd
                    dq.dma_start(out=out_d[qb * 4 + qs, :, dt * 512:(dt + 1) * 512],
                                 in_=ostg[:, dt * 512:(dt + 1) * 512])

            def mk_oproj(qb, qs, dt):
                def run():
                    if dt == 0:
                        oproj_states[qb, qs] = {"stacked": [stacked0[qb], stacked1[qb]]}
                    oproj_piece(qb, qs, dt)
                return run

            # ---- startup: c-tile-pipelined first projections ----
            with nc.named_scope("qkv"):
                pk0 = ps_misc.tile([128, 512], FP32, tag="misc", name="pk0")
                pq0 = ps_misc.tile([128, 512], FP32, tag="misc", name="pq0")
                for kt in range(4):
                    nc.tensor.matmul(pk0[:], wqk[:, kt, 2 * 128:3 * 128],
                                     xT_nt[0][:, kt, :],
                                     start=(kt == 0), stop=False)
                    nc.tensor.matmul(pq0[:], wqk[:, kt, 0:128],
                                     xT_nt[0][:, kt, :],
                                     start=(kt == 0), stop=False)
                # filler warmup while the second wqk/xT0 halves stream in,
                # so the HAM clock gate stays open for the c4-7 matmuls
                pw2 = ps_av.tile([128, 512], FP32, tag="pav", name="pw2")
                for r in range(8):
                    nc.tensor.matmul(pw2[:], wsrc[:, 0:128], wsrc[:],
                                     start=(r == 0), stop=(r == 7))
                # all k matmuls before q so the k rope chain starts earlier
                for kt in range(4, KT_D):
                    nc.tensor.matmul(pk0[:], wqk[:, kt, 2 * 128:3 * 128],
                                     xT_nt[0][:, kt, :],
                                     start=False, stop=(kt == KT_D - 1))
                for kt in range(4, KT_D):
                    nc.tensor.matmul(pq0[:], wqk[:, kt, 0:128],
                                     xT_nt[0][:, kt, :],
                                     start=False, stop=(kt == KT_D - 1))
                qk_rope(2, 0, pk0, "scalar")
                qk_rope(0, 0, pq0, "scalar")

            # ---- extras tables ----
            # P(m, nt) piece lists; v(tt); oproj(qb, qs, dt)
            P = {}
            for m, nt in [(2, 1), (2, 2), (2, 3), (0, 1), (0, 2), (0, 3),
                          (3, 0), (3, 1), (3, 2), (3, 3),
                          (1, 0), (1, 1), (1, 2), (1, 3)]:
                P[m, nt] = qk_proj_pieces(m, nt, "vector")

            def V(tt):
                return lambda: v_proj(tt)

            extras = [dict() for _ in range(8)]
            extras[0] = {
                0: [V(0)],
                1: [V(1), P[2, 1][0], P[2, 1][1]],
                2: [V(2), P[2, 1][2], P[2, 1][3]],
                3: [V(3)],
                4: [V(4), P[2, 2][0], P[2, 2][1]],
                5: [V(5), P[2, 2][2], P[2, 2][3]],
                6: [V(6)],
                7: [V(7)],
                8: [V(8), P[2, 3][0], P[2, 3][1]],
                9: [V(9), P[2, 3][2], P[2, 3][3]],
                10: [V(10)],
                11: [V(11), P[0, 1][0]],
                12: [V(12), P[0, 1][1]],
                13: [V(13), P[0, 1][2]],
                14: [V(14), P[0, 1][3]],
                15: [V(15)],
            }
            extras[1] = {2: [P[0, 2][0]], 3: [P[0, 2][1]], 4: [P[0, 2][2]], 5: [P[0, 2][3]]}
            extras[2] = {2: [P[0, 3][0]], 3: [P[0, 3][1]], 4: [P[0, 3][2]], 5: [P[0, 3][3]],
                         8: [P[3, 0][0]], 9: [P[3, 0][1]], 10: [P[3, 0][2]], 11: [P[3, 0][3]]}
            extras[3] = {0: [P[3, 1][0]], 1: [P[3, 1][1]], 2: [P[3, 1][2]], 3: [P[3, 1][3]],
                         4: [P[3, 2][0]], 5: [P[3, 2][1]], 6: [P[3, 2][2]], 7: [P[3, 2][3]],
                         8: [P[1, 0][0]], 9: [P[1, 0][1]], 10: [P[1, 0][2]], 11: [P[1, 0][3]],
                         12: [P[3, 3][0]], 13: [P[3, 3][1]], 14: [P[3, 3][2]], 15: [P[3, 3][3]]}
            extras[4] = {0: [P[1, 1][0]], 1: [P[1, 1][1]], 2: [P[1, 1][2]], 3: [P[1, 1][3]]}
            extras[5] = {0: [P[1, 2][0]], 1: [P[1, 2][1]], 2: [P[1, 2][2]], 3: [P[1, 2][3]],
                         5: [mk_oproj(0, 0, 0)], 6: [mk_oproj(0, 0, 1)],
                         7: [mk_oproj(0, 1, 0)], 8: [mk_oproj(0, 1, 1)],
                         9: [mk_oproj(0, 2, 0)], 10: [mk_oproj(0, 2, 1)],
                         11: [mk_oproj(0, 3, 0)], 12: [mk_oproj(0, 3, 1)]}
            extras[6] = {0: [P[1, 3][0]], 1: [P[1, 3][1]], 2: [P[1, 3][2]], 3: [P[1, 3][3]],
                         5: [mk_oproj(1, 0, 0)], 6: [mk_oproj(1, 0, 1)],
                         7: [mk_oproj(1, 1, 0)], 8: [mk_oproj(1, 1, 1)],
                         9: [mk_oproj(1, 2, 0)], 10: [mk_oproj(1, 2, 1)],
                         11: [mk_oproj(1, 3, 0)], 12: [mk_oproj(1, 3, 1)]}
            extras[7] = {4: [mk_oproj(2, 0, 0)], 5: [mk_oproj(2, 0, 1)],
                         6: [mk_oproj(2, 1, 0)], 7: [mk_oproj(2, 1, 1)],
                         8: [mk_oproj(2, 2, 0)], 9: [mk_oproj(2, 2, 1)],
                         10: [mk_oproj(2, 3, 0)], 11: [mk_oproj(2, 3, 1)]}

            # ---- pipelined block driver ----
            blocks = [(0, 0), (0, 1), (0, 2), (0, 3), (1, 0), (1, 1), (1, 2), (1, 3)]
            units = [make_block(p, qb) for (p, qb) in blocks]
            stacked0 = [None] * 4
            stacked1 = [None] * 4
            pending_fin = None
            units[0][0](0)  # sc_exp(block0, kt0)
            for bi in range(8):
                sc_exp, av, norm_front, norm_fin = units[bi]
                lag = 2
                for kt in range(KT_N):
                    # scores for the NEXT ktile go first: nothing that can
                    # stall (DMA-gated extras) may starve the exp stream
                    if kt < KT_N - 1:
                        sc_exp(kt + 1)
                    elif bi < 7:
                        units[bi + 1][0](0)  # next block's sc_exp(0)
                    if kt == 3 and pending_fin is not None:
                        pbi, fin = pending_fin
                        stknp = fin()
                        (stacked0 if pbi < 4 else stacked1)[pbi % 4] = stknp
                        pending_fin = None
                    if kt >= lag:
                        av(kt - lag)
                    for fn in extras[bi].get(kt, ()):
                        fn()
                for r in range(KT_N - lag, KT_N):
                    av(r)
                norm_front(last=(bi == 7))
                if bi == 7:
                    stacked1[3] = norm_fin()
                else:
                    pending_fin = (bi, norm_fin)

            # tail: out-projection of the last q-block (scalar engine is free
            # after the final exp, so evict through it)
            for qs in range(4):
                oproj_states[3, qs] = {"stacked": [stacked0[3], stacked1[3]]}
                oproj_piece(3, qs, 0, act_evict=True, alt_pool=True)
                oproj_piece(3, qs, 1, act_evict=True, alt_pool=True)

            ctx.close()

    nc.compile()
    return nc


def _host_prep(hidden_states, w_qkv):
    """Per-core input maps (host-side shard + layout prep)."""
    invf = 1.0 / (THETA ** (np.arange(0, DH, 2, dtype=np.float32) / DH))
    t = np.arange(N, dtype=np.float32)
    d_idx = np.arange(64)
    f = invf[(d_idx % 64) // 2]
    ang = t[None, :] * f[:, None]
    cos2 = np.cos(ang)
    sign = np.where(d_idx % 2 == 0, -1.0, 1.0).astype(np.float32)
    sin2n = np.sin(ang) * sign[:, None]
    cs = np.ascontiguousarray(np.stack([cos2, sin2n], axis=1)).astype(BF)

    # [NT, 128, KT_D, 512] partition-major so device DMAs are contiguous
    xT_b = [np.ascontiguousarray(
                hidden_states[b].T.astype(BF).reshape(D // 128, 128, N // 512, 512)
                .transpose(2, 1, 0, 3))
            for b in range(B)]

    in_maps = []
    for c in range(8):
        b, g = c // 4, c % 4
        heads = [4 * g, 4 * g + 1, 4 * g + 2, 4 * g + 3]
        cols = []
        for off in (0, 1024):
            for h in heads:
                cols.append(w_qkv[:, off + h * 64: off + (h + 1) * 64])
        wqk = np.ascontiguousarray(
            np.concatenate(cols, axis=1).astype(BF).reshape(D // 128, 128, 512).transpose(1, 0, 2))
        wv = np.ascontiguousarray(
            np.concatenate([w_qkv[:, 2048 + h * 64: 2048 + (h + 1) * 64] for h in heads],
                           axis=1).astype(BF).reshape(D // 128, 128, 256).transpose(1, 0, 2))
        in_maps.append({
            "xT": xT_b[b],
            "wqk": np.ascontiguousarray(wqk),
            "wv": wv,
            "cs": cs,
        })
    return in_maps


def kernel(hidden_states, w_qkv, w_out, b_out, _trace=False, _tmpdir=None):
    hidden_states = np.asarray(hidden_states, dtype=np.float32)
    w_qkv = np.asarray(w_qkv, dtype=np.float32)
    w_out = np.asarray(w_out, dtype=np.float32)
    b_out = np.asarray(b_out, dtype=np.float32)

    from concourse.bass_utils import run_bass_kernel_spmd

    if "nc" not in _CACHE:
        _CACHE["nc"] = _build()
    nc = _CACHE["nc"]

    in_maps = _host_prep(hidden_states, w_qkv)
    for c in range(8):
        g = c % 4
        wo = np.ascontiguousarray(
            w_out[4 * g * 64: 4 * g * 64 + 256, :].astype(BF).reshape(2, 128, 1024).transpose(1, 0, 2))
        in_maps[c]["wo"] = wo

    kwargs = {}
    if _trace:
        kwargs = dict(trace=True, tmpdir=_tmpdir)
    res = run_bass_kernel_spmd(nc, in_maps, core_ids=list(range(8)), **kwargs)

    out = np.zeros((B, N, D), dtype=np.float32)
    for c in range(8):
        out[c // 4] += res.results[c]["out"].reshape(N, D).astype(np.float32)
    out += b_out[None, None, :]
    if _trace:
        _CACHE["last_res"] = res
    return out
